# revision 47
# baseline (speedup 1.0000x reference)
"""Trainium2 Bass kernel for the NeuralMemory (Titans-style fast-weight) module.

Algorithm: the reference materializes per-token fast-weight matrices via
[L,L]x[L,H,D] einsums (~17 GFLOP). Every per-token gradient is rank-1
(outer product), so the whole recurrence collapses algebraically to
[L,L] decay/attention matrices and a handful of [128,256]x[256,256]
matmuls (~0.3 GFLOP). Derivation (per batch, L=128, D=H=256):

  q,k,v = x@W{q,k,v}+b;  Z1 = k@W1.T+b1; X2 = silu(Z1); Z2 = X2@W2.T+b2
  gZ2 = Z2-v; gZ1 = (gZ2@W2)*silu'(Z1)
  lr, log_mom, log_wd from x-projections; cm/cw = cumsum of logs
  M[l,m] = exp(cm[l]-cm[m]) (l>=m); Dm likewise with cw; C = Dm@M
  dcum = Dm@exp(cm); mom_cum/wd_cum = exp(cumsums)
  Zq1 = (C*lr*(q@k.T+1))@gZ1 + wd_cum*(q@W1.T+b1) - dcum*(q@mW1.T+mb1)
  Zq2 = (C*lr*(silu(Zq1)@X2.T+1))@gZ2 + wd_cum*(silu(Zq1)@W2.T+b2)
        - dcum*(silu(Zq1)@mW2.T+mb2)
  finals at l=L-1 are rank-L contractions over tokens plus scaled bases.

Only the {exp, ln} ACT table set is used (softplus = ln(1+exp),
sigmoid = 1/(1+exp(-x)) with DVE reciprocal) to avoid ACT table reloads.

Sharding: data-parallel over batch (B=2); each core runs one full batch
item (cores 0..7 -> batch core%2; results taken from cores 0,1).
Host work is layout-only: transposes/concats of weights, batch slicing.
"""

import numpy as np

import concourse.bass as bass
import concourse.tile as tile
from concourse import bacc
from concourse import mybir
from concourse.masks import make_identity

L = 128
D = 256
H = 256
F32 = mybir.dt.float32
F32R = mybir.dt.float32r
AF = mybir.ActivationFunctionType
ALU = mybir.AluOpType
LR_SHIFT = float(np.log(np.expm1(0.01)))

# wpack column offsets (per 128-row half of the K dimension)
WQ, WK, WV, S3 = 0, 256, 512, 768
W1T_, MW1T_, W2T_, MW2T_, W2N_ = 0, 256, 512, 768, 1024
HOTC = 771     # Wq|Wk|Wv|s3w
COLDC = 1280   # W1T|mW1T|W2T|mW2T|W2n


def _patch_act_tables(arch):
    """Prune the cached ACT-table map (indices preserved) so exp/ln resolve
    only to the combined natural_log_exp_and_others set -> single table load."""
    from concourse.hw_specs import get_activation_tables
    tabs = get_activation_tables(arch)
    keep = "natural_log_exp_and_others"
    if keep not in tabs:
        return
    for name, funcs in tabs.items():
        if name != keep:
            funcs.discard(AF.Exp)
            funcs.discard(AF.Ln)
            funcs.discard(AF.Identity)
            funcs.discard(AF.Copy)


def build_nc():
    nc = bacc.Bacc(None)
    _patch_act_tables(nc.m.arch)

    di = lambda name, shape, dt=F32R: nc.declare_dram_parameter(name, list(shape), dt, isOutput=False)
    do = lambda name, shape: nc.declare_dram_parameter(name, list(shape), F32, isOutput=True)

    d_xT = di("xT", [D, L])            # x[b].T
    d_hot0 = di("hot0", [128, HOTC])   # [Wq|Wk|Wv|s3w] rows 0:128
    d_hot1 = di("hot1", [128, HOTC])   # rows 128:256
    d_cold0 = di("cold0", [128, COLDC])  # [W1T|mW1T|W2T|mW2T|W2n] rows 0:128
    d_cold1 = di("cold1", [128, COLDC])
    d_bkv3 = di("bkv3", [1, 515])      # [bk|bv|bs3eff]  bs3eff=[blr+SHIFT,-bm,bd]
    d_brow4 = di("brow4", [1, 1024])   # [b1|mb1|b2|mb2]
    d_b1mb1 = di("b1mb1", [2, H])
    d_b2mb2 = di("b2mb2", [2, D])
    d_bcols = di("bcols", [128, 4], F32)    # [bq0|bq1|bk0|bk1] columns

    o_Zq2 = do("Zq2_o", [L, D])
    o_W0 = do("W0_o", [128, 1024])     # rows 0:128 of [W1pT|mgW1T|W2pT|mgW2T]
    o_W1 = do("W1_o", [128, 1024])     # rows 128:256
    o_b = do("b_o", [1, 1024])         # [b1p|mgb1|b2p|mgb2]

    with tile.TileContext(nc) as tc:
        with (
            tc.tile_pool(name="w", bufs=1) as wp,
            tc.tile_pool(name="psA", bufs=4, space=bass.MemorySpace.PSUM) as psA,
            tc.tile_pool(name="psB", bufs=2, space=bass.MemorySpace.PSUM) as psB,
            tc.tile_pool(name="psC", bufs=2, space=bass.MemorySpace.PSUM) as psC,
        ):
            def psmm(shape):
                return psA.tile(shape, F32, tag="mm", name="psmm")

            def pstp(shape):
                return psB.tile(shape, F32, tag="tp", name="pstp")

            def pssm(shape):
                return psC.tile(shape, F32, tag="sm", name="pssm")

            def sb(shape, tag, dtype=F32R):
                return wp.tile(shape, dtype, tag=tag, name=tag)

            def mmf(out, lhsT, rhs, **kw):
                # plain-fp32 matmul for tiny moving dims (fp32r ISA restriction)
                if kw.get("is_transpose"):
                    out = out.bitcast(F32)
                nc.tensor.matmul(out, lhsT.bitcast(F32), rhs.bitcast(F32), **kw)

            def mm(out, lhsT, rhs, **kw):
                # float32r: 4x faster moving-dim streaming than fp32 (N>=256)
                if kw.get("is_transpose"):
                    out = out.bitcast(F32R)
                nc.tensor.matmul(out, lhsT.bitcast(F32R), rhs.bitcast(F32R), **kw)

            # ---- constants (memset/affine_select need f32; converting
            # copies provide the f32r-rounded matmul views) ----
            identf = sb([128, 128], "identf", F32)
            make_identity(nc, identf)
            ident = sb([128, 128], "ident")
            nc.vector.tensor_copy(ident, identf)
            onesf = sb([128, 128], "onesf", F32)
            nc.gpsimd.memset(onesf[0:1, :], 1.0)
            nc.gpsimd.memset(onesf[:, 0:1], 1.0)
            ones_r = sb([1, 128], "ones_r")
            nc.vector.tensor_copy(ones_r, onesf[0:1, :])
            ones_c = sb([128, 1], "ones_c")
            nc.vector.tensor_copy(ones_c, onesf[:, 0:1])

            # ---- ACT table warmup: trigger the single {exp,ln} table load
            # at t~0 so it never sits on the critical path ----
            warm = sb([1, 1], "warm", F32)
            nc.vector.memset(warm, 0.0)
            nc.scalar.activation(warm, warm, AF.Exp)

            # ---- input DMAs: tiny biases first (scalar ring), then big packs
            # in consumption order (sync ring); W1T split out of cold so Z1
            # can start before the rest of the cold pack lands ----
            bkv3 = sb([1, 515], "bkv3"); nc.scalar.dma_start(out=bkv3, in_=d_bkv3[:])
            brow4 = sb([1, 1024], "brow4"); nc.scalar.dma_start(out=brow4, in_=d_brow4[:])
            b1mb1 = sb([2, H], "b1mb1"); nc.scalar.dma_start(out=b1mb1, in_=d_b1mb1[:])
            b2mb2 = sb([2, D], "b2mb2"); nc.scalar.dma_start(out=b2mb2, in_=d_b2mb2[:])
            bcols = sb([128, 4], "bcols", F32); nc.scalar.dma_start(out=bcols, in_=d_bcols[:])
            xT = sb([128, 2, 128], "xT")        # halves of x.T, [p, half, l]
            nc.sync.dma_start(out=xT, in_=d_xT.rearrange("(a p) l -> p a l", p=128))
            hot0 = sb([128, HOTC], "hot0"); nc.sync.dma_start(out=hot0, in_=d_hot0[:])
            hot1 = sb([128, HOTC], "hot1"); nc.sync.dma_start(out=hot1, in_=d_hot1[:])
            cold0 = sb([128, COLDC], "cold0")
            cold1 = sb([128, COLDC], "cold1")
            nc.sync.dma_start(out=cold0[:, 0:256], in_=d_cold0[:, 0:256])      # W1T rows 0:128
            nc.sync.dma_start(out=cold1[:, 0:256], in_=d_cold1[:, 0:256])
            nc.sync.dma_start(out=cold0[:, 256:COLDC], in_=d_cold0[:, 256:COLDC])
            nc.sync.dma_start(out=cold1[:, 256:COLDC], in_=d_cold1[:, 256:COLDC])

            xT0, xT1 = xT[:, 0], xT[:, 1]
            b1_r = brow4[:, 0:256]; mb1_r = brow4[:, 256:512]
            b2_r = brow4[:, 512:768]; mb2_r = brow4[:, 768:1024]

            # ================= q/k/v and MLP forward =================
            # qT computed directly transposed: lhsT = Wq cols, rhs = xT
            def qT_half(h):
                ps = pstp([128, 128])
                mm(ps, hot0[:, WQ + h * 128:WQ + (h + 1) * 128], xT0, start=True, stop=False)
                mm(ps, hot1[:, WQ + h * 128:WQ + (h + 1) * 128], xT1, start=False, stop=True)
                t = sb([128, 128], f"qT{h}", F32)
                nc.scalar.activation(t, ps, AF.Identity, bias=bcols[:, h:h + 1])
                return t

            qT0 = qT_half(0)
            qT1 = qT_half(1)

            # kT computed directly transposed (same trick as qT)
            def kT_half(h):
                ps = pstp([128, 128])
                mm(ps, hot0[:, WK + h * 128:WK + (h + 1) * 128], xT0, start=True, stop=False)
                mm(ps, hot1[:, WK + h * 128:WK + (h + 1) * 128], xT1, start=False, stop=True)
                t = sb([128, 128], f"kT{h}", F32)
                nc.vector.tensor_scalar(t, ps, bcols[:, 2 + h:3 + h], None, ALU.add)
                return t

            kT0 = kT_half(0)
            kT1 = kT_half(1)

            # v projection
            v_ps = psmm([L, D])
            mmf(v_ps, xT0, hot0[:, WV:WV + 256], start=True, stop=False)
            mmf(v_ps, xT1, hot1[:, WV:WV + 256], start=False, stop=False)
            mmf(v_ps, ones_r, bkv3[:, 256:512], start=False, stop=True)
            v_sb = sb([L, D], "v_sb", F32)
            nc.scalar.activation(v_sb, v_ps, AF.Identity)

            def pe_transpose(src_ap, tag, eng="v"):
                ps = pstp([128, 128])
                mmf(ps, src_ap, identf, is_transpose=True, start=True, stop=True)
                t = sb([128, 128], tag, F32)
                if eng == "v":
                    nc.vector.tensor_copy(t, ps)
                else:
                    nc.scalar.activation(t, ps, AF.Identity)
                return t

            # k natural (only needed by the late final-weight matmuls)
            k_sb = sb([L, D], "k_sb")
            for h, kt in ((0, kT0), (1, kT1)):
                psk = pstp([128, 128])
                mmf(psk, kt, identf, is_transpose=True, start=True, stop=True)
                nc.vector.tensor_copy(k_sb[:, h * 128:(h + 1) * 128], psk)

            Z1_ps = psmm([L, H])
            mmf(Z1_ps, kT0, cold0[:, W1T_:W1T_ + 256], start=True, stop=False)
            mmf(Z1_ps, kT1, cold1[:, W1T_:W1T_ + 256], start=False, stop=False)
            mmf(Z1_ps, ones_r, b1_r, start=False, stop=True)
            # sigmoid(Z1) via exp + reciprocal; silu and silu' composed
            eZ1 = sb([L, H], "eZ1", F32)
            nc.scalar.activation(eZ1, Z1_ps, AF.Exp, scale=-1.0)
            sp1 = sb([L, H], "sp1", F32)
            nc.vector.tensor_scalar_add(sp1, eZ1, 1.0)
            sg1 = sb([L, H], "sg1", F32)
            nc.vector.reciprocal(sg1, sp1)
            X2_sb = sb([L, H], "X2_sb", F32)
            nc.vector.tensor_mul(X2_sb, Z1_ps, sg1)
            dsA = sb([L, H], "dsA", F32)
            nc.gpsimd.tensor_add(dsA, X2_sb, sg1)
            dsB = sb([L, H], "dsB", F32)
            nc.gpsimd.tensor_mul(dsB, sg1, X2_sb)
            dS1_sb = sb([L, H], "dS1_sb", F32)
            nc.gpsimd.tensor_sub(dS1_sb, dsA, dsB)

            X2T0 = pe_transpose(X2_sb[:, 0:128], "X2T0", "v")
            X2T1 = pe_transpose(X2_sb[:, 128:256], "X2T1", "a")

            Z2_ps = psmm([L, D])
            mmf(Z2_ps, X2T0, cold0[:, W2T_:W2T_ + 256], start=True, stop=False)
            mmf(Z2_ps, X2T1, cold1[:, W2T_:W2T_ + 256], start=False, stop=False)
            mmf(Z2_ps, ones_r, b2_r, start=False, stop=True)
            gZ2_sb = sb([L, D], "gZ2_sb", F32)
            nc.vector.tensor_sub(gZ2_sb, Z2_ps, v_sb)

            gZ2T0 = pe_transpose(gZ2_sb[:, 0:128], "gZ2T0", "v")
            gZ2T1 = pe_transpose(gZ2_sb[:, 128:256], "gZ2T1", "a")

            gX2_ps = psmm([L, H])
            mmf(gX2_ps, gZ2T0, cold0[:, W2N_:W2N_ + 256], start=True, stop=False)
            mmf(gX2_ps, gZ2T1, cold1[:, W2N_:W2N_ + 256], start=False, stop=True)
            gZ1_sb = sb([L, H], "gZ1_sb")
            nc.vector.tensor_mul(gZ1_sb, gX2_ps, dS1_sb)

            # ================= per-token scalars & decay path =================
            # all three scalar projections in one [1,384] psum; host negates
            # the Wm column so every softplus is ln(1+exp(raw + bias)); the
            # scalar biases fold in as K=1 matmuls -> a single Exp+Ln pair.
            s3_ps = pssm([1, 384])
            for i in range(3):
                sl = s3_ps[:, i * 128:(i + 1) * 128]
                mm(sl, hot0[:, S3 + i:S3 + i + 1], xT0, start=True, stop=False, skip_group_check=True)
                mm(sl, hot1[:, S3 + i:S3 + i + 1], xT1, start=False, stop=False, skip_group_check=True)
                mm(sl, bkv3[:, 512 + i:513 + i], ones_r, start=False, stop=True, skip_group_check=True)
            e3 = sb([1, 384], "e3", F32)
            nc.scalar.activation(e3, s3_ps, AF.Exp)
            sp3 = sb([1, 384], "sp3")   # [softplus_lr | -log_mom | -log_wd]
            nc.scalar.activation(sp3, e3, AF.Ln, bias=1.0, scale=1.0)
            lr_r = sp3[:, 0:128]
            nlg_m = sp3[:, 128:256]
            nlg_w = sp3[:, 256:384]

            ncmw = sb([1, 256], "ncmw", F32)   # [-cumsum(log_mom) | -cumsum(log_wd)]
            ncm_r = ncmw[:, 0:128]
            ncw_r = ncmw[:, 128:256]
            nc.vector.tensor_tensor_scan(ncm_r, nlg_m, nlg_m, 0.0, ALU.add, ALU.bypass)
            nc.vector.tensor_tensor_scan(ncw_r, nlg_w, nlg_w, 0.0, ALU.add, ALU.bypass)

            mw_cum_r = sb([1, 256], "mw_cum_r", F32)
            nc.scalar.activation(mw_cum_r, ncmw, AF.Exp, scale=-1.0)
            mom_cum_r = mw_cum_r[:, 0:128]
            wd_cum_r = mw_cum_r[:, 128:256]

            def row2col(src_row, tag):
                ps = pssm([128, 1])
                mmf(ps, src_row, identf[0:1, 0:1], is_transpose=True, start=True, stop=True)
                t = sb([128, 1], tag, F32)
                nc.vector.tensor_copy(t, ps)
                return t

            lr_c = row2col(lr_r, "lr_c")
            ncm_c = row2col(ncm_r, "ncm_c")
            ncw_c = row2col(ncw_r, "ncw_c")
            neg_ncm_c = sb([128, 1], "neg_ncm_c", F32)
            nc.vector.tensor_scalar_mul(neg_ncm_c, ncm_c, -1.0)
            mom_cum_c = sb([128, 1], "mom_cum_c", F32)
            nc.scalar.activation(mom_cum_c, ncm_c, AF.Exp, scale=-1.0)
            wd_cum_c = sb([128, 1], "wd_cum_c", F32)
            nc.scalar.activation(wd_cum_c, ncw_c, AF.Exp, scale=-1.0)

            # decay tiles: arg built on DVE, masked on gpsimd, exp'd on ACT
            bc_ps = pssm([128, 256])
            mmf(bc_ps, onesf[0:1, :], ncmw, start=True, stop=True)
            bc_m_ps = bc_ps[:, 0:128]
            bc_w_ps = bc_ps[:, 128:256]

            def decay_tile(bc_ps, col_ap, sign_in, keep_p_ge_f, tag):
                t = sb([128, 128], tag)
                if sign_in > 0:
                    nc.vector.tensor_scalar(t, bc_ps, col_ap, None, ALU.add)
                else:
                    nc.vector.tensor_scalar(t, bc_ps, -1.0, col_ap, ALU.mult, ALU.add)
                if keep_p_ge_f:
                    nc.gpsimd.affine_select(out=t, in_=t, compare_op=ALU.is_ge,
                                            fill=-87.0, base=0,
                                            pattern=[[-1, 128]], channel_multiplier=1)
                else:
                    nc.gpsimd.affine_select(out=t, in_=t, compare_op=ALU.is_ge,
                                            fill=-87.0, base=0,
                                            pattern=[[1, 128]], channel_multiplier=-1)
                nc.scalar.activation(t, t, AF.Exp)
                return t

            Mst = decay_tile(bc_m_ps, neg_ncm_c, +1, True, "Mst")    # M[m,n], keep m>=n
            M2 = decay_tile(bc_m_ps, ncm_c, -1, False, "M2")         # M[l,n] at [n,l]
            Dt = decay_tile(bc_w_ps, ncw_c, -1, False, "Dt")         # Dm[l,m] at [m,l]

            CT_ps = psmm([128, 128])
            mm(CT_ps, Mst, Dt, start=True, stop=True)   # CT[n,l] = C[l,n]
            CLt = sb([128, 128], "CLt", F32)
            nc.vector.tensor_scalar_mul(CLt, CT_ps, lr_c)

            Dsc = sb([128, 128], "Dsc")
            nc.vector.tensor_scalar_mul(Dsc, Dt, mom_cum_c)
            dcum_c_ps = pssm([128, 1])
            mmf(dcum_c_ps, Dsc, onesf[:, 0:1], start=True, stop=True)
            dcum_r_ps = pssm([1, 128])
            mmf(dcum_r_ps, onesf[:, 0:1], Dsc, start=True, stop=True)
            ndcum_c = sb([128, 1], "ndcum_c", F32)
            nc.vector.tensor_scalar_mul(ndcum_c, dcum_c_ps, -1.0)
            dcum_r = sb([1, 128], "dcum_r", F32)
            nc.vector.tensor_copy(dcum_r, dcum_r_ps)

            # last-token scalars -> one broadcast matmul [128,3]
            lasts_r = sb([1, 3], "lasts_r")
            nc.vector.tensor_copy(lasts_r[:, 0:1], wd_cum_r[:, 127:128])
            nc.vector.tensor_copy(lasts_r[:, 1:2], mom_cum_r[:, 127:128])
            nc.vector.tensor_copy(lasts_r[:, 2:3], dcum_r[:, 127:128])
            lasts_ps = pssm([128, 3])
            mmf(lasts_ps, onesf[0:1, :], lasts_r, start=True, stop=True)
            lasts = sb([128, 3], "lasts", F32)
            nc.vector.tensor_copy(lasts, lasts_ps)
            wd_last_c = lasts[:, 0:1]
            nlasts = sb([128, 2], "nlasts", F32)   # [-mom_last | -dcum_last]
            nc.vector.tensor_scalar_mul(nlasts, lasts[:, 1:3], -1.0)
            nmom_last_c = nlasts[:, 0:1]
            ndcum_last_c = nlasts[:, 1:2]
            nl2 = sb([1, 2], "nl2")   # f32r copy for K=1 matmul lhsT use
            nc.vector.tensor_copy(nl2, nlasts[0:1, :])

            # wdnd rows for the K=2 bias matmuls: [wd_cum | -dcum] as [2,128]
            wdnd_c = sb([128, 2], "wdnd_c")
            nc.vector.tensor_copy(wdnd_c[:, 0:1], wd_cum_c)
            nc.vector.tensor_copy(wdnd_c[:, 1:2], ndcum_c)
            wdnd_ps = pssm([2, 128])
            mm(wdnd_ps, wdnd_c, ident, is_transpose=True, start=True, stop=True)
            wdnd = sb([2, 128], "wdnd")
            nc.vector.tensor_copy(wdnd, wdnd_ps)

            # scaled identities: diag(wd_last), diag(-dcum_last), diag(-mom_last)
            sidw = sb([128, 128], "sidw")
            nc.vector.tensor_scalar_mul(sidw, ident, wd_last_c)
            sidnd = sb([128, 128], "sidnd")
            nc.vector.tensor_scalar_mul(sidnd, ident, ndcum_last_c)
            sidnm = sb([128, 128], "sidnm")
            nc.gpsimd.tensor_scalar_mul(sidnm, ident, nmom_last_c)

            # [wd_cum | -dcum] broadcast rows for pre-scaling qT/sq1T columns
            wnd_row = sb([1, 256], "wnd_row")
            nc.vector.tensor_copy(wnd_row[:, 0:128], wd_cum_r)
            nc.vector.tensor_scalar_mul(wnd_row[:, 128:256], dcum_r, -1.0)
            wnd_bc_ps = pssm([128, 256])
            mm(wnd_bc_ps, ones_r, wnd_row, start=True, stop=True)
            wnd_bc = sb([128, 256], "wnd_bc")
            nc.vector.tensor_copy(wnd_bc, wnd_bc_ps)
            wd_bc = wnd_bc[:, 0:128]
            nd_bc = wnd_bc[:, 128:256]

            # pre-scaled qT columns: qTw = wd_cum[l]*qT, qTn = -dcum[l]*qT
            qTw0 = sb([128, 128], "qTw0"); nc.vector.tensor_mul(qTw0, qT0, wd_bc)
            qTw1 = sb([128, 128], "qTw1"); nc.gpsimd.tensor_mul(qTw1, qT1, wd_bc)
            qTn0 = sb([128, 128], "qTn0"); nc.vector.tensor_mul(qTn0, qT0, nd_bc)
            qTn1 = sb([128, 128], "qTn1"); nc.gpsimd.tensor_mul(qTn1, qT1, nd_bc)

            # ================= layer-1 query pass =================
            ST_ps = psmm([128, 128])
            mmf(ST_ps, kT0, qT0, start=True, stop=False)
            mmf(ST_ps, kT1, qT1, start=False, stop=True)
            G1 = sb([128, 128], "G1")
            nc.vector.scalar_tensor_tensor(G1, ST_ps, 1.0, CLt, ALU.add, ALU.mult)

            # Zq1 entirely in one PSUM group:
            # G1term + [wd|-dcum] x [b1|mb1] + (wd*q)@W1T + (-dcum*q)@mW1T
            Zq1_ps = psmm([L, H])
            mm(Zq1_ps, G1, gZ1_sb, start=True, stop=False)
            mm(Zq1_ps, wdnd, b1mb1, start=False, stop=False)
            mm(Zq1_ps, qTw0, cold0[:, 0:256], start=False, stop=False)
            mm(Zq1_ps, qTw1, cold1[:, 0:256], start=False, stop=False)
            mm(Zq1_ps, qTn0, cold0[:, 256:512], start=False, stop=False)
            mm(Zq1_ps, qTn1, cold1[:, 256:512], start=False, stop=True)
            Zq1_sb = sb([L, H], "Zq1_sb", F32)
            nc.scalar.activation(Zq1_sb, Zq1_ps, AF.Identity)
            eZq1 = sb([L, H], "eZq1", F32)
            nc.scalar.activation(eZq1, Zq1_ps, AF.Exp, scale=-1.0)
            spq = sb([L, H], "spq", F32)
            nc.vector.tensor_scalar_add(spq, eZq1, 1.0)
            sgq = sb([L, H], "sgq", F32)
            nc.vector.reciprocal(sgq, spq)
            sq1_sb = sb([L, H], "sq1_sb", F32)
            nc.vector.tensor_mul(sq1_sb, Zq1_sb, sgq)

            # ================= final-token outputs (overlap layer-2) =================
            cw_c = sb([128, 1], "cw_c", F32)
            nc.vector.tensor_copy(cw_c, CLt[:, 127:128])
            cm_c = sb([128, 1], "cm_c", F32)
            nc.vector.tensor_mul(cm_c, M2[:, 127:128].bitcast(F32), lr_c)

            gcs1 = sb([L, 512], "gcs1")   # [gZ1*cw | gZ1*cm]
            nc.vector.tensor_scalar_mul(gcs1[:, 0:256], gZ1_sb, cw_c)
            nc.gpsimd.tensor_scalar_mul(gcs1[:, 256:512], gZ1_sb, cm_c)
            X2r = sb([L, H], "X2r")       # f32r view of X2 for the final matmuls
            nc.gpsimd.tensor_copy(X2r, X2_sb)
            gcs2 = sb([L, 512], "gcs2")   # [gZ2*cw | gZ2*cm]
            nc.vector.tensor_scalar_mul(gcs2[:, 0:256], gZ2_sb, cw_c)
            nc.gpsimd.tensor_scalar_mul(gcs2[:, 256:512], gZ2_sb, cm_c)

            wo0 = sb([128, 1024], "wo0", F32)   # [W1pT|mgW1T|W2pT|mgW2T] rows 0:128
            wo1 = sb([128, 1024], "wo1", F32)
            bo = sb([1, 1024], "bo", F32)       # [b1p|mgb1|b2p|mgb2]

            def wfinal(lhs_sb, gcs, base_off, col_off, n):
                # full result accumulated in PSUM: token-sum + scaled bases
                for h, wo in ((0, wo0), (1, wo1)):
                    ps = psmm([128, 2 * n])
                    base0 = (cold0, cold1)[h]
                    mm(ps, lhs_sb[:, h * 128:(h + 1) * 128], gcs,
                       start=True, stop=False, skip_group_check=True)
                    mm(ps[:, 0:n], sidw, base0[:, base_off:base_off + n],
                       start=False, stop=False, skip_group_check=True)
                    mm(ps[:, 0:n], sidnd, base0[:, base_off + n:base_off + 2 * n],
                       start=False, stop=False, skip_group_check=True)
                    mm(ps[:, n:2 * n], sidnm, base0[:, base_off + n:base_off + 2 * n],
                       start=False, stop=True, skip_group_check=True)
                    nc.scalar.activation(wo[:, col_off:col_off + 2 * n], ps, AF.Identity)

            # W1pT/mgW1T: lhsT = k halves, rhs = gcs1; bases W1T/mW1T at cold[:,0:512]
            wfinal(k_sb, gcs1, W1T_, 0, 256)
            # W2pT/mgW2T: lhsT = X2 halves, rhs = gcs2; bases W2T/mW2T at cold[:,512:1024]
            wfinal(X2r, gcs2, W2T_, 512, 256)

            def vfinal(gcs, base_r, mom_r, col_off, n):
                ps = pssm([1, 2 * n])
                mm(ps, ones_c, gcs, start=True, stop=False, skip_group_check=True)
                mm(ps[:, 0:n], lasts_r[:, 0:1], base_r,
                   start=False, stop=False, skip_group_check=True)     # +wd_last*b
                mm(ps[:, 0:n], nl2[:, 1:2], mom_r,
                   start=False, stop=False, skip_group_check=True)     # -dcum_last*mb
                mm(ps[:, n:2 * n], nl2[:, 0:1], mom_r,
                   start=False, stop=True, skip_group_check=True)      # -mom_last*mb
                nc.vector.tensor_copy(bo[:, col_off:col_off + 2 * n], ps)

            vfinal(gcs1, b1_r, mb1_r, 0, 256)
            vfinal(gcs2, b2_r, mb2_r, 512, 256)

            nc.scalar.dma_start(out=o_W0[:], in_=wo0)
            nc.scalar.dma_start(out=o_W1[:], in_=wo1)
            nc.sync.dma_start(out=o_b[:], in_=bo)

            # ================= layer-2 query pass =================
            sq1T0 = pe_transpose(sq1_sb[:, 0:128], "sq1T0", "v")
            sq1T1 = pe_transpose(sq1_sb[:, 128:256], "sq1T1", "a")

            TT_ps = psmm([128, 128])
            mmf(TT_ps, X2T0, sq1T0, start=True, stop=False)
            mmf(TT_ps, X2T1, sq1T1, start=False, stop=True)
            G2 = sb([128, 128], "G2")
            nc.vector.scalar_tensor_tensor(G2, TT_ps, 1.0, CLt, ALU.add, ALU.mult)

            # pre-scaled sq1T columns
            sq1Tw0 = sb([128, 128], "sq1Tw0"); nc.vector.tensor_mul(sq1Tw0, sq1T0, wd_bc)
            sq1Tw1 = sb([128, 128], "sq1Tw1"); nc.gpsimd.tensor_mul(sq1Tw1, sq1T1, wd_bc)
            sq1Tn0 = sb([128, 128], "sq1Tn0"); nc.vector.tensor_mul(sq1Tn0, sq1T0, nd_bc)
            sq1Tn1 = sb([128, 128], "sq1Tn1"); nc.gpsimd.tensor_mul(sq1Tn1, sq1T1, nd_bc)

            gZ2r = sb([L, D], "gZ2r")     # f32r view of gZ2 for the Zq2 matmul
            nc.gpsimd.tensor_copy(gZ2r, gZ2_sb)
            Zq2_ps = psmm([L, D])
            mm(Zq2_ps, G2, gZ2r, start=True, stop=False)
            mm(Zq2_ps, wdnd, b2mb2, start=False, stop=False)
            mm(Zq2_ps, sq1Tw0, cold0[:, 512:768], start=False, stop=False)
            mm(Zq2_ps, sq1Tw1, cold1[:, 512:768], start=False, stop=False)
            mm(Zq2_ps, sq1Tn0, cold0[:, 768:1024], start=False, stop=False)
            mm(Zq2_ps, sq1Tn1, cold1[:, 768:1024], start=False, stop=True)
            Zq2_sb = sb([L, D], "Zq2_sb", F32)
            nc.vector.tensor_copy(Zq2_sb, Zq2_ps)
            nc.sync.dma_start(out=o_Zq2[:], in_=Zq2_sb)

    nc.compile()
    return nc


_NC = None


def _get_nc():
    global _NC
    if _NC is None:
        _NC = build_nc()
    return _NC


def _in_map(inp, b):
    f = np.float32
    c = np.ascontiguousarray

    def half(w, h):
        return w[h * 128:(h + 1) * 128].astype(f)

    s3w = np.concatenate([inp["Wlr"], -inp["Wm"], inp["Wd"]], axis=1).astype(f)  # [256,3]
    W1T = inp["W1"][b].astype(f).T
    mW1T = inp["mW1"][b].astype(f).T
    W2T = inp["W2"][b].astype(f).T
    mW2T = inp["mW2"][b].astype(f).T
    W2n = inp["W2"][b].astype(f)
    hot = np.concatenate([inp["Wq"].astype(f), inp["Wk"].astype(f),
                          inp["Wv"].astype(f), s3w], axis=1)        # [256, 771]
    cold = np.concatenate([W1T, mW1T, W2T, mW2T, W2n], axis=1)      # [256, 1280]
    bs3eff = np.array([[inp["blr"][0] + LR_SHIFT, -inp["bm"][0], inp["bd"][0]]], f)
    bkv3 = np.concatenate([inp["bk"].astype(f)[None, :], inp["bv"].astype(f)[None, :], bs3eff], axis=1)
    brow4 = np.concatenate([inp["b1"][b], inp["mb1"][b], inp["b2"][b], inp["mb2"][b]]).astype(f)[None, :]
    return {
        "xT": c(inp["x"][b].astype(f).T),
        "hot0": c(hot[0:128]), "hot1": c(hot[128:256]),
        "cold0": c(cold[0:128]), "cold1": c(cold[128:256]),
        "bkv3": c(bkv3), "brow4": c(brow4),
        "b1mb1": c(np.stack([inp["b1"][b], inp["mb1"][b]]).astype(f)),
        "b2mb2": c(np.stack([inp["b2"][b], inp["mb2"][b]]).astype(f)),
        "bcols": c(np.concatenate([inp["bq"].astype(f).reshape(2, 128).T,
                                   inp["bk"].astype(f).reshape(2, 128).T], axis=1)),
    }


def _assemble(res01):
    Zq2 = np.stack([r["Zq2_o"] for r in res01])

    def wmat(r, off):
        full = np.concatenate([r["W0_o"][:, off:off + 256], r["W1_o"][:, off:off + 256]], axis=0)
        return full.T

    W1p = np.stack([wmat(r, 0) for r in res01])
    mgW1 = np.stack([wmat(r, 256) for r in res01])
    W2p = np.stack([wmat(r, 512) for r in res01])
    mgW2 = np.stack([wmat(r, 768) for r in res01])
    b1p = np.stack([r["b_o"][0, 0:256] for r in res01])
    mgb1 = np.stack([r["b_o"][0, 256:512] for r in res01])
    b2p = np.stack([r["b_o"][0, 512:768] for r in res01])
    mgb2 = np.stack([r["b_o"][0, 768:1024] for r in res01])
    return (Zq2, W1p, b1p, W2p, b2p, mgW1, mgb1, mgW2, mgb2)


def kernel(**inputs):
    from concourse.bass_utils import run_bass_kernel_spmd

    nc = _get_nc()
    in_maps = [_in_map(inputs, core % 2) for core in range(8)]
    out = run_bass_kernel_spmd(nc, in_maps, list(range(8)))
    return _assemble(out.results[:2])
